# revision 9
# baseline (speedup 1.0000x reference)
"""Masked attention (B=4, M=N=4096, D=64) on 8 Trainium2 NeuronCores.

Sharding: batch (4) x m-halves (2) -> 8 cores, no cross-core communication.
Each core computes out[m, :] = softmax(mask(q@k^T)/sqrt(d)) @ v for its
2048 q rows against the full 4096 k/v rows of its batch.

v5: four-engine elementwise split.  Scores are computed transposed in
PSUM pair-tiles S [128n, 1024] = (even|odd) n-chunk x 512 m.  Per
n-pair flavor:

  A: ScalarE exp + DVE mult by the fp16 notmask (2x mode).
  B: PE accumulates -240*maskT into the scores via an fp8 identity
     matmul pre-exp (masked -> e^-30*e^s -> 0 in fp16); exp only.
  F: DVE Schraudolph fast-exp (int16 round of a*S+b written through a
     bitcast = the fp16 encoding of exp(s/8-3), +-3% sawtooth that
     softmax normalization averages out; worst-core rel err of this mix
     is simulated offline at 1.49e-2 vs the 2e-2 gate) + GPSIMD mult
     by the notmask.

PV runs over m-block pairs so the two PV matmuls of a chunk share one
LDWEIGHTS.  Mask DMAs are batched per (n-pair, block-pair) and laid out
[sub][e|o][m] so every consumer slice is contiguous.  q/k DMAs are split
so the first QK only waits on ~128KB.
"""

import numpy as np
import ml_dtypes
from contextlib import ExitStack

import concourse.bacc as bacc
import concourse.mybir as mybir
import concourse.tile as tile
from concourse.bass_utils import run_bass_kernel_spmd

B, M, N, D = 4, 4096, 4096, 64
NCORES = 8
M_LOC = M // 2        # q rows per core
MH = 512              # m sub-block of one scores tile column-half
NMB = M_LOC // MH     # 4 m-blocks
NBP = NMB // 2        # 2 m-block-pairs
NCH = N // 128        # 32 n-chunks of 128
NPAIR = NCH // 2      # 16 chunk-pairs
SCALE = 1.0 / 8.0     # 1/sqrt(64)
EBIAS = -3.0
MASKC = 240.0         # fp8 mask subtractor: exp sees s/8 - 30 -> 0 in fp16
LOG2E = 1.4426950408889634
FE_A = SCALE * 1024.0 * LOG2E                  # fast-exp scale on raw s
FE_B = 1024.0 * (15.0 + EBIAS * LOG2E) - 44.0  # fp16-bits bias, centered
BF16 = mybir.dt.bfloat16
F32 = mybir.dt.float32
FP16 = mybir.dt.float16
FP8 = mybir.dt.float8e4
I16 = mybir.dt.int16

# pair flavor schedule (16 n-pairs, same at every m-block):
# A=exp+DVEmult, B=maskadd+exp, F=fastexp+GPSIMDmult.
PAIR_TYPES = ["A", "F", "A", "A", "A", "B", "F", "A",
              "A", "A", "F", "A", "A", "A", "B", "F"]
assert len(PAIR_TYPES) == NPAIR

_NC = None
LAST_RESULTS = None   # BassKernelResults of the most recent run (for profiling)
TRACE = False
TRACE_KW = {}
_RUN_IDX = 0


def _build_nc():
    nc = bacc.Bacc("TRN2", target_bir_lowering=False, debug=False,
                   num_devices=NCORES)
    qT = nc.dram_tensor("qT", [128, M_LOC], FP16, kind="ExternalInput").ap()
    kT = nc.dram_tensor("kT", [128, NPAIR * 128], FP16,
                        kind="ExternalInput").ap()
    vA = nc.dram_tensor("vA", [128, NCH * (D + 1)], FP16,
                        kind="ExternalInput").ap()
    # notmask, pre-arranged per (n-pair, block-pair) as [128][sub][e|o][m]
    nmT = nc.dram_tensor("nmT", [NPAIR, NBP, 128, 4 * MH], FP16,
                         kind="ExternalInput").ap()
    m8T = nc.dram_tensor("m8T", [NPAIR, NBP, 128, 4 * MH], FP8,
                         kind="ExternalInput").ap()
    id8 = nc.dram_tensor("id8", [128, 128], FP8, kind="ExternalInput").ap()
    # raw accumulator output: out^T with the softmax denominator in row 64;
    # the host does the (tiny) divide + transpose during unsharding
    o = nc.dram_tensor("oT", [NBP, D + 1, 2 * MH], F32,
                       kind="ExternalOutput").ap()

    with tile.TileContext(nc) as tc, ExitStack() as ctx:
        const = ctx.enter_context(tc.tile_pool(name="const", bufs=1))
        m16pool = ctx.enter_context(tc.tile_pool(name="m16", bufs=4))
        m8pool = ctx.enter_context(tc.tile_pool(name="m8", bufs=2))
        epool = ctx.enter_context(tc.tile_pool(name="e", bufs=4))
        ppool = ctx.enter_context(tc.tile_pool(name="p", bufs=8))
        fpool = ctx.enter_context(tc.tile_pool(name="fin", bufs=2))
        spool = ctx.enter_context(tc.tile_pool(name="spsum", bufs=3, space="PSUM"))
        opool = ctx.enter_context(tc.tile_pool(name="opsum", bufs=1, space="PSUM"))

        # constants, split so the first QK waits on as little DMA as
        # possible; spread over the sync/scalar/gpsimd queues.
        kT_s = const.tile([128, NPAIR * 128], FP16)
        nc.scalar.dma_start(kT_s[:, 0:512], kT[:, 0:512])
        qT_s = const.tile([128, M_LOC], FP16)
        nc.sync.dma_start(qT_s[:, 0:MH], qT[:, 0:MH])
        nc.gpsimd.dma_start(qT_s[:, MH:2 * MH], qT[:, MH:2 * MH])
        nc.scalar.dma_start(kT_s[:, 512:NPAIR * 128], kT[:, 512:NPAIR * 128])
        nc.sync.dma_start(qT_s[:, 2 * MH:3 * MH], qT[:, 2 * MH:3 * MH])
        nc.gpsimd.dma_start(qT_s[:, 3 * MH:4 * MH], qT[:, 3 * MH:4 * MH])
        id8_s = const.tile([128, 128], FP8)
        nc.sync.dma_start(id8_s[:], id8)
        vA_s = const.tile([128, NCH * (D + 1)], FP16)
        nc.scalar.dma_start(vA_s[:], vA)
        ebias = const.tile([128, 1], F32)
        nc.vector.memset(ebias[:], EBIAS)
        # warmup operand with no DMA dependency (starts right after preamble)
        wsrc = const.tile([128, 512], BF16)
        nc.vector.memset(wsrc[:], 1.0)

        # a few dense K=128 matmuls bridge the PE from preamble-end to the
        # first QK (and start warming the HAM clock gate)
        wu = spool.tile([128, 2 * MH], F32, tag="s")
        for _ in range(5):
            nc.tensor.matmul(wu[:, 0:512], wsrc[:, 0:128], wsrc[:],
                             start=True, stop=True)

        for bp in range(NBP):
            o_ps = opool.tile([D + 1, 2 * MH], F32)
            pv_pending = []

            def flush_pv():
                # two PV matmuls per chunk (m-blocks 2bp, 2bp+1) sharing
                # one LDWEIGHTS of the vA chunk
                for ni, pr0, pr1 in pv_pending:
                    vch = vA_s[:, ni * (D + 1):(ni + 1) * (D + 1)]
                    nc.tensor.matmul(o_ps[:, 0:MH], vch, pr0,
                                     start=(ni == 0), stop=(ni == NCH - 1))
                    nc.tensor.matmul(o_ps[:, MH:2 * MH], vch, pr1,
                                     start=(ni == 0), stop=(ni == NCH - 1))
                pv_pending.clear()

            for pc in range(NPAIR):
                pt = PAIR_TYPES[pc]
                ni_e, ni_o = 2 * pc, 2 * pc + 1
                lhs_e = kT_s[0:64, pc * 128:(pc + 1) * 128]
                lhs_o = kT_s[64:128, pc * 128:(pc + 1) * 128]
                # QK for both sub-blocks first (adjacent packed pairs)
                Ss = []
                for sub in range(2):
                    mb = 2 * bp + sub
                    rhs_e = qT_s[0:64, mb * MH:(mb + 1) * MH]
                    rhs_o = qT_s[64:128, mb * MH:(mb + 1) * MH]
                    S = spool.tile([128, 2 * MH], F32, tag="s")
                    qk_stop = pt != "B"
                    nc.tensor.matmul(S[:, 0:MH], lhs_e, rhs_e,
                                     start=True, stop=qk_stop,
                                     tile_position=(0, 0))
                    nc.tensor.matmul(S[:, MH:2 * MH], lhs_o, rhs_o,
                                     start=True, stop=qk_stop,
                                     tile_position=(64, 0))
                    Ss.append(S)
                    if sub == 0:
                        # PV of the PREVIOUS pair between this pair's QKs
                        flush_pv()
                ps = []
                if pt == "B":
                    m8 = m8pool.tile([128, 4 * MH], FP8)
                    nc.sync.dma_start(m8[:], m8T[pc, bp])
                    for sub in range(2):
                        half = m8[:, sub * 2 * MH:(sub + 1) * 2 * MH]
                        nc.tensor.matmul(Ss[sub][:, 0:MH], id8_s[:],
                                         half[:, 0:MH], start=False, stop=True)
                        nc.tensor.matmul(Ss[sub][:, MH:2 * MH], id8_s[:],
                                         half[:, MH:2 * MH],
                                         start=False, stop=True)
                    for sub in range(2):
                        p = ppool.tile([128, 2 * MH], FP16)
                        nc.scalar.activation(p[:], Ss[sub][:],
                                             mybir.ActivationFunctionType.Exp,
                                             bias=ebias[:], scale=SCALE)
                        ps.append(p)
                else:
                    nm = m16pool.tile([128, 4 * MH], FP16)
                    nc.sync.dma_start(nm[:], nmT[pc, bp])
                    for sub in range(2):
                        e = epool.tile([128, 2 * MH], FP16)
                        if pt == "A":
                            nc.scalar.activation(
                                e[:], Ss[sub][:],
                                mybir.ActivationFunctionType.Exp,
                                bias=ebias[:], scale=SCALE)
                        else:  # F: DVE fast-exp via int16 bitcast
                            nc.vector.tensor_scalar(
                                e[:].bitcast(I16), Ss[sub][:], FE_A, FE_B,
                                mybir.AluOpType.mult, mybir.AluOpType.add)
                        p = ppool.tile([128, 2 * MH], FP16)
                        nmh = nm[:, sub * 2 * MH:(sub + 1) * 2 * MH]
                        if pt == "A":
                            nc.vector.tensor_mul(p[:], e[:], nmh)
                        else:
                            nc.gpsimd.tensor_mul(p[:], e[:], nmh)
                        ps.append(p)
                pv_pending.append((ni_e, ps[0][:, 0:MH], ps[1][:, 0:MH]))
                pv_pending.append((ni_o, ps[0][:, MH:2 * MH],
                                   ps[1][:, MH:2 * MH]))
            flush_pv()
            oT = fpool.tile([D + 1, 2 * MH], F32)
            nc.vector.tensor_copy(oT[:], o_ps[:])
            nc.sync.dma_start(o[bp], oT[:])
    nc.compile()
    return nc


def _get_nc():
    global _NC
    if _NC is None:
        _NC = _build_nc()
    return _NC


_ID8 = None


def _arrange_masks(mT16, mT8):
    """[n, m] -> [NPAIR, NBP, 128, 4*MH] with layout [sub][e|o][m]."""
    def arr(x):
        # x: [4096, 2048] -> pairs of chunks (e,o), m-block pairs (sub)
        r = x.reshape(NPAIR, 2, 128, NBP, 2, MH)     # [pc, t, p, bp, sub, m]
        return np.ascontiguousarray(
            r.transpose(0, 3, 2, 4, 1, 5).reshape(NPAIR, NBP, 128, 4 * MH))
    return arr(mT16), arr(mT8)


def _prep_core(q, k, v, mask, b, j):
    global _ID8
    qs = q[b, j * M_LOC:(j + 1) * M_LOC, :]
    qT = np.ascontiguousarray(qs.T).astype(np.float16)    # [64, 2048]
    qTp = np.concatenate([qT, qT], axis=0)                # [128, 2048]
    kTf = np.ascontiguousarray(k[b].T).astype(np.float16) # [64, 4096]
    kTp = np.empty((128, NPAIR * 128), np.float16)
    kTr = kTf.reshape(64, NCH, 128)
    kTp[0:64] = kTr[:, 0::2, :].reshape(64, -1)
    kTp[64:128] = kTr[:, 1::2, :].reshape(64, -1)
    vb = v[b]                                             # [4096, 64]
    vA = np.empty((128, NCH * (D + 1)), np.float16)
    vAr = vA.reshape(128, NCH, D + 1)
    vAr[:, :, :D] = vb.reshape(NCH, 128, D).transpose(1, 0, 2).astype(np.float16)
    vAr[:, :, D] = np.float16(1.0)
    mT = np.ascontiguousarray(mask[b, j * M_LOC:(j + 1) * M_LOC, :].T)
    nmT, m8T = _arrange_masks((~mT).astype(np.float16),
                              mT.astype(ml_dtypes.float8_e4m3))
    if _ID8 is None:
        _ID8 = (np.eye(128, dtype=np.float32) * -MASKC).astype(
            ml_dtypes.float8_e4m3)
    return {"qT": qTp, "kT": kTp, "vA": vA, "nmT": nmT, "m8T": m8T,
            "id8": _ID8}


def kernel(q, k, v, mask):
    global LAST_RESULTS, _RUN_IDX
    q = np.asarray(q, dtype=np.float32)
    k = np.asarray(k, dtype=np.float32)
    v = np.asarray(v, dtype=np.float32)
    mask = np.asarray(mask)
    nc = _get_nc()
    in_maps = [_prep_core(q, k, v, mask, c // 2, c % 2) for c in range(NCORES)]
    kw = dict(TRACE_KW)
    if "tmpdir" in kw:
        import os
        _RUN_IDX += 1
        kw["tmpdir"] = os.path.join(kw["tmpdir"], f"run{_RUN_IDX}")
        os.makedirs(kw["tmpdir"], exist_ok=True)
    res = run_bass_kernel_spmd(nc, in_maps, core_ids=list(range(NCORES)),
                               trace=TRACE, **kw)
    LAST_RESULTS = res
    out = np.empty((B, M, D), np.float32)
    for c in range(NCORES):
        b, j = divmod(c, 2)
        oT = res.results[c]["oT"]                      # [NBP, 65, 2*MH]
        for bp in range(NBP):
            for sub in range(2):
                blk = (oT[bp, :D, sub * MH:(sub + 1) * MH] /
                       oT[bp, D, sub * MH:(sub + 1) * MH])
                lo = j * M_LOC + (2 * bp + sub) * MH
                out[b, lo:lo + MH, :] = blk.T
    return out
